# revision 19
# baseline (speedup 1.0000x reference)
"""Trainium2 Bass kernel for nn_DownsampleTransformerBlock.

Self-contained: takes FULL inputs (as from setup_inputs()), shards across 8
NeuronCores (batch x query-half data parallel), runs one SPMD Bass/Tile
kernel, and reassembles the full output (down [B,K,D], idx [B,K]) on host.

Sharding: core c handles batch b=c//2, query half c%2 (1024 queries).
MHA2 needs full-sequence out1 for K/V -> one pairwise AllGather mid-kernel.
Importance colsums: exact-fp32 main term via ACT-accumulate during exp,
plus a small correction term (per-row softmax normalization deviation from
1/S) computed from bf16 attention probs via DVE scalar_tensor_tensor.
"""
import os
import sys
import numpy as np

for _p in ("/opt/trn_rl_repo", "/root/.axon_site"):
    if _p not in sys.path:
        sys.path.insert(0, _p)

from contextlib import ExitStack

import concourse.bass as bass
import concourse.tile as tile
from concourse import bacc, mybir
from concourse.bass_utils import run_bass_kernel_spmd

F32 = mybir.dt.float32
BF16 = mybir.dt.bfloat16
ALU = mybir.AluOpType
AF = mybir.ActivationFunctionType

B, S, D, H, DK, F = 4, 2048, 256, 8, 32, 1024
Q = S // 2              # queries per core
NST = S // 128          # 16 s-tiles
NQT = Q // 128          # 8 q-tiles
SCALE = float(1.0 / np.sqrt(DK))
RBAR = float(1.0 / S)
LN_EPS = 1e-6

_CACHE = {}
LAST_EXEC_NS = None


def _build():
    nc = bacc.Bacc("TRN2", target_bir_lowering=False, debug=False,
                   num_devices=8)

    # ---- I/O ----
    x_full = nc.dram_tensor("x_full", [S, D], F32, kind="ExternalInput")
    xq_in = nc.dram_tensor("xq", [Q, D], F32, kind="ExternalInput")
    ident_in = nc.dram_tensor("ident", [128, 128], F32, kind="ExternalInput")
    dram_in = {}
    for p in ("g", "l"):
        for t in ("q", "k", "v", "o"):
            dram_in[f"{p}{t}_w"] = nc.dram_tensor(f"{p}{t}_w", [D, D], F32,
                                                  kind="ExternalInput")
            dram_in[f"{p}{t}_b"] = nc.dram_tensor(f"{p}{t}_b", [D], F32,
                                                  kind="ExternalInput")
    for nm, shp, dt in (("ffn_w1", [D, F], F32), ("ffn_b1", [F], F32),
                        ("ffn_w2bf", [F, D], BF16), ("ffn_b2", [D], F32),
                        ("res_w", [D, D], F32), ("res_b", [D], F32),
                        ("ln1_g", [D], F32), ("ln1_b", [D], F32),
                        ("ln2_g", [D], F32), ("ln2_b", [D], F32),
                        ("ln3_g", [D], F32), ("ln3_b", [D], F32),
                        ("bnM", [D], F32), ("bnC", [D], F32)):
        dram_in[nm] = nc.dram_tensor(nm, shp, dt, kind="ExternalInput")
    o_bn = nc.dram_tensor("o_bn", [D, Q], F32, kind="ExternalOutput")
    o_colg = nc.dram_tensor("o_colg", [S], F32, kind="ExternalOutput")
    o_coll = nc.dram_tensor("o_coll", [S], F32, kind="ExternalOutput")

    with tile.TileContext(nc) as tc, ExitStack() as ctx:
        const = ctx.enter_context(tc.tile_pool(name="const", bufs=1))
        acts = ctx.enter_context(tc.tile_pool(name="acts", bufs=1))
        rows = ctx.enter_context(tc.tile_pool(name="rows", bufs=1))
        small = ctx.enter_context(tc.tile_pool(name="small", bufs=2))
        work = ctx.enter_context(tc.tile_pool(name="work", bufs=2))
        epool = ctx.enter_context(tc.tile_pool(name="epool", bufs=2))
        ps = ctx.enter_context(tc.tile_pool(name="ps", bufs=1, space="PSUM"))
        dram = ctx.enter_context(tc.tile_pool(name="dram", bufs=1, space="DRAM"))

        def rows_via_dram(dst_row, src_pn, ntiles):
            """dst_row [1, ntiles*128] <- src_pn [128, ntiles] with s=n*128+p."""
            drs = dram.tile([ntiles * 128], F32, tag="drs", bufs=2)
            nc.gpsimd.dma_start(drs[:].rearrange("(n p) -> p n", p=128), src_pn)
            nc.gpsimd.dma_start(dst_row, drs[:].rearrange("(o s) -> o s", o=1))

        # ---- constants ----
        ident = rows.tile([128, 128], F32, tag="bvb")
        nc.sync.dma_start(ident, ident_in[:, :])
        ones_row = const.tile([1, 128], F32)
        nc.vector.memset(ones_row, 1.0)
        ones_col = const.tile([128, 1], F32)
        nc.vector.memset(ones_col, 1.0)
        ones_row32 = const.tile([33, 128], F32)
        nc.vector.memset(ones_row32[32:33, :], 1.0)

        def load_mat(name, rows_, cols, tag, pool=const, dt=F32):
            t = pool.tile([128, rows_ // 128, cols], dt, tag=tag)
            nc.sync.dma_start(t, dram_in[name].rearrange("(c p) n -> p c n",
                                                         p=128))
            return t

        def load_col(name, n=D, tag=None):
            t = const.tile([128, n // 128], F32, tag=tag or f"c_{name}")
            nc.sync.dma_start(t, dram_in[name].rearrange("(c p) -> p c", p=128))
            return t

        def load_row32(name, n=D, tag=None):
            t = const.tile([33, n], F32, tag=tag or f"r_{name}")
            nc.sync.dma_start(t[32:33, :],
                              dram_in[name].rearrange("(o n) -> o n", o=1))
            return t

        w_sb = {}
        for nm in ("ffn_b1", "ffn_b2", "res_b", "ln1_g", "ln1_b", "ln2_g",
                   "ln2_b", "ln3_g", "ln3_b", "bnM", "bnC"):
            w_sb[nm] = load_col(nm, F if nm == "ffn_b1" else D)
        for nm in ("ln1_g", "ln2_g", "ln3_g"):
            w_sb[nm + "_row"] = load_row32(nm)

        # ---- load x (token-major) ----
        xf_tok = acts.tile([128, NST, D], F32, tag="tokA")
        nc.sync.dma_start(xf_tok, x_full.rearrange("(n p) d -> p n d", p=128))
        xq_tok = acts.tile([128, NQT, D], F32, tag="midA")
        nc.sync.dma_start(xq_tok, xq_in.rearrange("(n p) d -> p n d", p=128))

        def transpose_to_feat(tok, ntiles, tag):
            ft = acts.tile([128, 2, ntiles * 128], F32, tag=tag)
            for i in range(ntiles):
                for dch in range(2):
                    pst = ps.tile([128, 128], F32, tag="scps", bufs=2)
                    nc.tensor.transpose(pst, tok[:, i, dch * 128:(dch + 1) * 128],
                                        ident)
                    nc.scalar.activation(ft[:, dch, i * 128:(i + 1) * 128], pst,
                                         AF.Identity)
            return ft

        xT = transpose_to_feat(xf_tok, NST, "bigA")
        xqT = transpose_to_feat(xq_tok, NQT, "xqT")

        # ---- LN helpers ----
        def rsqrt_dve(v_eps, shape):
            y = small.tile(list(shape), F32, tag="rsq_y")
            a = small.tile(list(shape), F32, tag="rsq_a")
            c = small.tile(list(shape), F32, tag="rsq_c")
            nc.vector.reciprocal(y, v_eps)
            for _ in range(4):
                nc.vector.tensor_tensor(out=a, in0=y, in1=y, op=ALU.mult)
                nc.vector.tensor_tensor(out=a, in0=a, in1=v_eps, op=ALU.mult)
                nc.vector.tensor_scalar(out=c, in0=a, scalar1=-0.5, scalar2=1.5,
                                        op0=ALU.mult, op1=ALU.add)
                nc.vector.tensor_tensor(out=y, in0=y, in1=c, op=ALU.mult)
            return y

        def stats_rows_from_tok(tok, ntiles):
            """token-major [128, n, 256] -> mr_rows [2, n*128] (m; rstd)."""
            stats = small.tile([128, ntiles, 6], F32, tag="ln_st")
            mv = small.tile([128, ntiles, 2], F32, tag="ln_mv")
            for i in range(ntiles):
                nc.vector.bn_stats(out=stats[:, i, :], in_=tok[:, i, :])
                nc.vector.bn_aggr(out=mv[:, i, :], in_=stats[:, i, :])
            veps = small.tile([128, ntiles], F32, tag="ln_ve")
            nc.vector.tensor_scalar(out=veps, in0=mv[:, :, 1], scalar1=LN_EPS,
                                    scalar2=None, op0=ALU.add)
            rstd = rsqrt_dve(veps, (128, ntiles))
            mrc = small.tile([128, ntiles], F32, tag="ln_mc")
            nc.vector.tensor_tensor(out=mrc, in0=mv[:, :, 0], in1=rstd,
                                    op=ALU.mult)
            mrr = rows.tile([33, ntiles * 128], F32, tag="mr_rows")
            rows_via_dram(mrr[0:1, :], rstd, ntiles)
            rows_via_dram(mrr[32:33, :], mrc, ntiles)
            return mrr

        def stats_rows_from_feat(ft, ntok):
            """feature-major [128, 2, ntok] -> mr_rows [33, ntok]
            (row 0 = rstd, row 32 = m*rstd)."""
            n = ntok // 128
            dsx = dram.tile([ntok], F32, tag="drs", bufs=2)
            dsxx = dram.tile([ntok], F32, tag="drs", bufs=2)
            for j in range(0, ntok, 512):
                pj = ps.tile([1, 512], F32, tag="scps", bufs=2)
                pk = ps.tile([1, 512], F32, tag="pvps")
                for dch in range(2):
                    nc.tensor.matmul(pj, ones_col, ft[:, dch, j:j + 512],
                                     start=(dch == 0), stop=(dch == 1))
                    sq = work.tile([128, 512], F32, tag="ln_sq", bufs=1)
                    nc.vector.tensor_tensor(out=sq, in0=ft[:, dch, j:j + 512],
                                            in1=ft[:, dch, j:j + 512],
                                            op=ALU.mult)
                    nc.tensor.matmul(pk, ones_col, sq,
                                     start=(dch == 0), stop=(dch == 1))
                prow = work.tile([1, 512], F32, tag="prow", bufs=1)
                nc.vector.tensor_copy(prow, pj)
                nc.gpsimd.dma_start(dsx[j:j + 512].rearrange("(o s) -> o s", o=1),
                                    prow)
                prow2 = work.tile([1, 512], F32, tag="prow2", bufs=1)
                nc.vector.tensor_copy(prow2, pk)
                nc.gpsimd.dma_start(dsxx[j:j + 512].rearrange("(o s) -> o s", o=1),
                                    prow2)
            sxr = small.tile([128, n], F32, tag="ln_fst")
            sxxr = small.tile([128, n], F32, tag="ln_fst2")
            nc.gpsimd.dma_start(sxr, dsx[:].rearrange("(n p) -> p n", p=128))
            nc.gpsimd.dma_start(sxxr, dsxx[:].rearrange("(n p) -> p n", p=128))
            m = small.tile([128, n], F32, tag="ln_m2")
            nc.vector.tensor_scalar(out=m, in0=sxr, scalar1=1.0 / D,
                                    scalar2=None, op0=ALU.mult)
            veps = small.tile([128, n], F32, tag="ln_ve")
            msq = small.tile([128, n], F32, tag="ln_msq")
            nc.vector.tensor_tensor(out=msq, in0=m, in1=m, op=ALU.mult)
            nc.vector.scalar_tensor_tensor(out=veps, in0=sxxr,
                                           scalar=1.0 / D, in1=msq,
                                           op0=ALU.mult, op1=ALU.subtract)
            nc.vector.tensor_scalar(out=veps, in0=veps, scalar1=LN_EPS,
                                    scalar2=None, op0=ALU.add)
            rstd = rsqrt_dve(veps, (128, n))
            mrc = small.tile([128, n], F32, tag="ln_mc")
            nc.vector.tensor_tensor(out=mrc, in0=m, in1=rstd, op=ALU.mult)
            mrr = rows.tile([33, ntok], F32, tag="mr_rows")
            rows_via_dram(mrr[0:1, :], rstd, n)
            rows_via_dram(mrr[32:33, :], mrc, n)
            return mrr

        def ln_apply(srcT, ntok, mrr, gkey, tag):
            """out = (x - m) * rstd * g + b, feature-major [128, 2, ntok]."""
            outT = acts.tile([128, 2, ntok], F32, tag=tag)
            g_col = w_sb[gkey + "_g"]
            b_col = w_sb[gkey + "_b"]
            g_row = w_sb[gkey + "_g_row"]
            for dch in range(2):
                for j in range(0, ntok, 512):
                    rb = ps.tile([128, 512], F32, tag="scps", bufs=2)
                    nc.tensor.matmul(rb, ones_row, mrr[0:1, j:j + 512],
                                     start=True, stop=True)
                    gmr = ps.tile([128, 512], F32, tag="pvps")
                    nc.tensor.matmul(gmr,
                                     g_row[32:33, dch * 128:(dch + 1) * 128],
                                     mrr[32:33, j:j + 512],
                                     start=True, stop=True, tile_position=(32, 0))
                    nc.vector.scalar_tensor_tensor(
                        out=outT[:, dch, j:j + 512], in0=srcT[:, dch, j:j + 512],
                        scalar=g_col[:, dch:dch + 1], in1=rb,
                        op0=ALU.mult, op1=ALU.mult)
                    nc.vector.scalar_tensor_tensor(
                        out=outT[:, dch, j:j + 512], in0=outT[:, dch, j:j + 512],
                        scalar=b_col[:, dch:dch + 1], in1=gmr,
                        op0=ALU.add, op1=ALU.subtract)
            return outT

        mr1f = stats_rows_from_tok(xf_tok, NST)
        norm1T = ln_apply(xT, S, mr1f, "ln1", "bigB")
        mr1q = stats_rows_from_tok(xq_tok, NQT)
        norm1qT = ln_apply(xqT, Q, mr1q, "ln1", "normqT")

        # ---- projections ----
        def proj_feat(normT, ntok, w_t, b_col, tag, pool=acts):
            out = pool.tile([128, 2, ntok], F32, tag=tag)
            for m in range(2):
                for j in range(0, ntok, 512):
                    psm = ps.tile([128, 512], F32, tag="scps", bufs=2)
                    for c in range(2):
                        nc.tensor.matmul(psm, w_t[:, c, m * 128:(m + 1) * 128],
                                         normT[:, c, j:j + 512],
                                         start=(c == 0), stop=(c == 1))
                    nc.scalar.activation(out[:, m, j:j + 512], psm, AF.Identity,
                                         bias=b_col[:, m:m + 1])
            return out

        def proj_v_aug(normT, w_t, bname):
            """V token-major with ones column: [128, NST, H, DK+2] bf16."""
            vaug = acts.tile([128, NST, H, DK + 2], BF16, tag="tokA")
            nc.vector.memset(vaug[:, :, :, DK:DK + 1], 1.0)
            bvb = rows.tile([128, D], F32, tag="bvb")
            bv_ap = dram_in[bname][:]
            nc.gpsimd.dma_start(
                bvb, bass.AP(tensor=bv_ap.tensor, offset=bv_ap.offset,
                             ap=[[0, 128], [1, D]]))
            for i in range(NST):
                psm = ps.tile([128, D], F32, tag="scps", bufs=2)
                for c in range(2):
                    nc.tensor.matmul(psm, normT[:, c, i * 128:(i + 1) * 128],
                                     w_t[:, c, :], start=(c == 0), stop=(c == 1))
                nc.vector.tensor_tensor(
                    out=vaug[:, i, :, 0:DK],
                    in0=psm.rearrange("p (h k) -> p h k", k=DK),
                    in1=bvb.rearrange("p (h k) -> p h k", k=DK), op=ALU.add)
            return vaug

        # ---- attention ----
        def mha(normT_full, normT_q, pfx, colsum_dst, res_src, res_dst):
            wq = load_mat(f"{pfx}q_w", D, D, "w_q")
            wk = load_mat(f"{pfx}k_w", D, D, "w_k")
            wv = load_mat(f"{pfx}v_w", D, D, "w_v")
            wo = load_mat(f"{pfx}o_w", D, D, "w_o")
            bq = load_col(f"{pfx}q_b", tag="b_q")
            bk = load_col(f"{pfx}k_b", tag="b_k")
            bo = load_col(f"{pfx}o_b", tag="b_o")
            ktT = proj_feat(normT_full, S, wk, bk, "bigA")
            qtT = proj_feat(normT_q, Q, wq, bq, "bigC")
            vaug = proj_v_aug(normT_full, wv, f"{pfx}v_b")

            colA = small.tile([128, NST], F32, tag="colA")
            colD = small.tile([128, NST], F32, tag="colD")
            otst = acts.tile([128, 2, Q], F32, tag="bigB")

            for h in range(H):
                pt, band = h // 4, (h % 4) * 32
                e_h = epool.tile([128, NST, Q], BF16, tag="e_h")
                a_h = small.tile([128, NST], F32, tag="a_h")
                pvps = ps.tile([DK + 1, Q], F32, tag="pvps")
                for st in range(NST):
                    scps = ps.tile([128, Q], F32, tag="scps", bufs=2)
                    for j in range(0, Q, 512):
                        nc.tensor.matmul(
                            scps[:, j:j + 512],
                            ktT[band:band + 32, pt, st * 128:(st + 1) * 128],
                            qtT[band:band + 32, pt, j:j + 512],
                            start=True, stop=True, tile_position=(band, 0))
                    nc.scalar.activation(e_h[:, st, :], scps, AF.Exp,
                                         scale=SCALE, accum_out=a_h[:, st:st + 1])
                    for j in range(0, Q, 512):
                        nc.tensor.matmul(pvps[:, j:j + 512],
                                         vaug[:, st, h, 0:DK + 1],
                                         e_h[:, st, j:j + 512],
                                         start=(st == 0), stop=(st == NST - 1))
                # r rows, O normalize, delta colsum
                ar = rows.tile([33, Q], F32, tag="mr_rows")
                nc.vector.reciprocal(ar[32:33, :], pvps[DK:DK + 1, :])
                osb = work.tile([DK, Q], F32, tag="osb", bufs=1)
                nc.scalar.activation(osb, pvps[0:DK, :], AF.Identity)
                bc = ps.tile([128, Q], F32, tag="bc")
                for j in range(0, Q, 512):
                    nc.tensor.matmul(bc[:, j:j + 512], ones_row32[32:33, :],
                                     ar[32:33, j:j + 512], start=True, stop=True,
                                     tile_position=(32, 0))
                otmp = work.tile([DK, Q], F32, tag="otmp", bufs=1)
                nc.vector.tensor_tensor(out=otmp, in0=osb, in1=bc[0:DK, :],
                                        op=ALU.mult)
                nc.sync.dma_start(otst[band:band + 32, pt, :], otmp)
                nc.vector.tensor_scalar(out=ar[32:33, :], in0=ar[32:33, :],
                                        scalar1=-RBAR, scalar2=None, op0=ALU.add)
                bc2 = ps.tile([128, Q], F32, tag="bc")
                for j in range(0, Q, 512):
                    nc.tensor.matmul(bc2[:, j:j + 512], ones_row32[32:33, :],
                                     ar[32:33, j:j + 512], start=True,
                                     stop=True, tile_position=(32, 0))
                scrap = work.tile([128, Q], BF16, tag="att_scrap", bufs=1)
                dcol = small.tile([128, NST], F32, tag="att_dcol")
                for st in range(NST):
                    nc.vector.scalar_tensor_tensor(
                        out=scrap, in0=e_h[:, st, :], scalar=1.0, in1=bc2,
                        op0=ALU.mult, op1=ALU.mult,
                        accum_out=dcol[:, st:st + 1])
                if h == 0:
                    nc.vector.tensor_copy(colA, a_h)
                    nc.vector.tensor_copy(colD, dcol)
                else:
                    nc.vector.tensor_tensor(out=colA, in0=colA, in1=a_h,
                                            op=ALU.add)
                    nc.vector.tensor_tensor(out=colD, in0=colD, in1=dcol,
                                            op=ALU.add)

            for m in range(2):
                for j in range(0, Q, 512):
                    psm = ps.tile([128, 512], F32, tag="scps", bufs=2)
                    for c in range(2):
                        nc.tensor.matmul(psm, wo[:, c, m * 128:(m + 1) * 128],
                                         otst[:, c, j:j + 512],
                                         start=(c == 0), stop=(c == 1))
                    nc.vector.scalar_tensor_tensor(
                        out=res_dst[:, m, j:j + 512], in0=psm,
                        scalar=bo[:, m:m + 1], in1=res_src[:, m, j:j + 512],
                        op0=ALU.add, op1=ALU.add)
            colsum = small.tile([128, NST], F32, tag="col_tot")
            nc.vector.tensor_scalar(out=colsum, in0=colA, scalar1=RBAR,
                                    scalar2=None, op0=ALU.mult)
            nc.vector.tensor_tensor(out=colsum, in0=colsum, in1=colD, op=ALU.add)
            nc.sync.dma_start(colsum_dst[:].rearrange("(n p) -> p n", p=128),
                              colsum)

        out1qT = acts.tile([128, 2, Q], F32, tag="midA")
        mha(norm1T, norm1qT, "g", o_colg, xqT, out1qT)

        # ---- AllGather out1 halves ----
        ag_in = dram.tile([D, Q], F32)
        ag_out = dram.tile([2, D, Q], F32)
        nc.sync.dma_start(ag_in.rearrange("(c p) q -> p c q", p=128), out1qT)
        nc.gpsimd.collective_compute(
            "AllGather", ALU.bypass,
            replica_groups=[[0, 1], [2, 3], [4, 5], [6, 7]],
            ins=[ag_in.opt()], outs=[ag_out.opt()])
        out1T = acts.tile([128, 2, S], F32, tag="bigA")
        for half in range(2):
            for dch in range(2):
                nc.sync.dma_start(out1T[:, dch, half * Q:(half + 1) * Q],
                                  ag_out[half, dch * 128:(dch + 1) * 128, :])

        mr2f = stats_rows_from_feat(out1T, S)
        norm2T = ln_apply(out1T, S, mr2f, "ln2", "bigB")
        mr2q = stats_rows_from_feat(out1qT, Q)
        norm2qT = ln_apply(out1qT, Q, mr2q, "ln2", "normqT")

        out2qT = acts.tile([128, 2, Q], F32, tag="bigC2")
        mha(norm2T, norm2qT, "l", o_coll, out1qT, out2qT)

        mr3q = stats_rows_from_feat(out2qT, Q)
        norm3qT = ln_apply(out2qT, Q, mr3q, "ln3", "normqT")

        # ---- FFN ----
        w1 = load_mat("ffn_w1", D, F, "bigA")
        b1 = w_sb["ffn_b1"]
        f1T = acts.tile([128, F // 128, Q], BF16, tag="tokA")
        for m in range(F // 128):
            for j in range(0, Q, 512):
                psm = ps.tile([128, 512], F32, tag="scps", bufs=2)
                for c in range(2):
                    nc.tensor.matmul(psm, w1[:, c, m * 128:(m + 1) * 128],
                                     norm3qT[:, c, j:j + 512],
                                     start=(c == 0), stop=(c == 1))
                nc.scalar.activation(f1T[:, m, j:j + 512], psm, AF.Relu,
                                     bias=b1[:, m:m + 1])
        w2_bf = load_mat("ffn_w2bf", F, D, "w_q", dt=BF16)
        b2 = w_sb["ffn_b2"]
        res_w = load_mat("res_w", D, D, "w_o")
        res_b = w_sb["res_b"]
        for dch in range(2):
            for j in range(0, Q, 512):
                psm = ps.tile([128, 512], F32, tag="scps", bufs=2)
                for c in range(F // 128):
                    nc.tensor.matmul(psm, w2_bf[:, c, dch * 128:(dch + 1) * 128],
                                     f1T[:, c, j:j + 512],
                                     start=(c == 0), stop=(c == F // 128 - 1))
                nc.vector.scalar_tensor_tensor(
                    out=out2qT[:, dch, j:j + 512], in0=psm,
                    scalar=b2[:, dch:dch + 1],
                    in1=out2qT[:, dch, j:j + 512], op0=ALU.add, op1=ALU.add)
                psr = ps.tile([128, 512], F32, tag="pvps")
                for c in range(2):
                    nc.tensor.matmul(psr, res_w[:, c, dch * 128:(dch + 1) * 128],
                                     xqT[:, c, j:j + 512],
                                     start=(c == 0), stop=(c == 1))
                resb = work.tile([128, 512], F32, tag="resb", bufs=1)
                nc.scalar.activation(resb, psr, AF.Identity,
                                     bias=res_b[:, dch:dch + 1])
                nc.vector.tensor_tensor(out=out2qT[:, dch, j:j + 512],
                                        in0=out2qT[:, dch, j:j + 512],
                                        in1=resb, op=ALU.add)
            nc.vector.tensor_scalar(out=out2qT[:, dch, :],
                                    in0=out2qT[:, dch, :],
                                    scalar1=w_sb["bnM"][:, dch:dch + 1],
                                    scalar2=w_sb["bnC"][:, dch:dch + 1],
                                    op0=ALU.mult, op1=ALU.add)
            nc.sync.dma_start(o_bn[dch * 128:(dch + 1) * 128, :],
                              out2qT[:, dch, :])

    nc.compile()
    return nc


def kernel(**inputs):
    global LAST_EXEC_NS
    inp = inputs
    x = np.asarray(inp["x"], np.float32)

    if "nc" not in _CACHE:
        _CACHE["nc"] = _build()
    nc = _CACHE["nc"]

    bnM = (np.asarray(inp["bn_g"], np.float32)
           * (1.0 / np.sqrt(np.asarray(inp["bn_var"], np.float32) + 1e-3))
           ).astype(np.float32)
    bnC = (np.asarray(inp["bn_b"], np.float32)
           - np.asarray(inp["bn_mean"], np.float32) * bnM).astype(np.float32)

    import ml_dtypes
    common = {"ident": np.eye(128, dtype=np.float32), "bnM": bnM, "bnC": bnC}
    for p in ("g", "l"):
        for t in ("q", "k", "v", "o"):
            common[f"{p}{t}_w"] = np.ascontiguousarray(
                np.asarray(inp[f"{p}{t}_w"], np.float32).reshape(D, D))
            common[f"{p}{t}_b"] = np.ascontiguousarray(
                np.asarray(inp[f"{p}{t}_b"], np.float32).reshape(D))
    for nm in ("ffn_w1", "ffn_b1", "ffn_b2", "res_w", "res_b",
               "ln1_g", "ln1_b", "ln2_g", "ln2_b", "ln3_g", "ln3_b"):
        common[nm] = np.ascontiguousarray(np.asarray(inp[nm], np.float32))
    common["ffn_w2bf"] = np.ascontiguousarray(
        np.asarray(inp["ffn_w2"], np.float32).astype(ml_dtypes.bfloat16))

    in_maps = []
    for c in range(8):
        b, half = c // 2, c % 2
        m = dict(common)
        m["x_full"] = np.ascontiguousarray(x[b])
        m["xq"] = np.ascontiguousarray(x[b, half * Q:(half + 1) * Q])
        in_maps.append(m)

    trace = bool(int(os.environ.get("DTB_TRACE", "0")))
    if trace:
        try:
            sys.path.insert(0, "/root/problem/work")
            import ntff_shim
            ntff_shim.install()
        except Exception:
            trace = False
    res = run_bass_kernel_spmd(nc, in_maps, list(range(8)), trace=trace,
                               trace_cores=list(range(8)) if trace else None)
    LAST_EXEC_NS = res.exec_time_ns

    colg = np.zeros((B, S), np.float32)
    coll = np.zeros((B, S), np.float32)
    bn_full = np.zeros((B, S, D), np.float32)
    for c in range(8):
        b, half = c // 2, c % 2
        out = res.results[c]
        colg[b] += out["o_colg"]
        coll[b] += out["o_coll"]
        bn_full[b, half * Q:(half + 1) * Q] = out["o_bn"].T

    imp = (np.float32(0.7 / (H * S)) * colg
           + np.float32(0.3 / (H * S)) * coll).astype(np.float32)
    imp = (imp / (imp.sum(-1, keepdims=True, dtype=np.float32)
                  + np.float32(1e-10))).astype(np.float32)
    si = np.argsort(-imp, axis=-1, kind="stable").astype(np.int32)
    sv = np.take_along_axis(imp, si, axis=-1)
    cum = np.cumsum(sv, axis=-1, dtype=np.float32)
    num = (cum <= np.float32(0.99)).sum(-1)
    min_pts = int(round(S * 0.1))
    num = np.minimum(np.maximum(num, min_pts), S)
    k = int(num.max())
    idx = np.sort(si[:, :k], axis=-1).astype(np.int32)
    down = np.take_along_axis(bn_full, idx[..., None], axis=1)
    return down, idx


# revision 26
# speedup vs baseline: 1.3052x; 1.3052x over previous
"""Trainium2 Bass kernel for nn_DownsampleTransformerBlock.

Self-contained: takes FULL inputs (as from setup_inputs()), shards across 8
NeuronCores (batch x query-half data parallel), runs one SPMD Bass/Tile
kernel, and reassembles the full output (down [B,K,D], idx [B,K]) on host.

Sharding: core c handles batch b=c//2, query half c%2 (1024 queries).
MHA2 needs full-sequence out1 for K/V -> one pairwise AllGather mid-kernel.
Importance colsums: exact-fp32 main term via ACT-accumulate during exp,
plus a small correction term (per-row softmax normalization deviation from
1/S) computed from bf16 attention probs via DVE scalar_tensor_tensor.
"""
import os
import sys
import numpy as np

for _p in ("/opt/trn_rl_repo", "/root/.axon_site"):
    if _p not in sys.path:
        sys.path.insert(0, _p)

from contextlib import ExitStack

import concourse.bass as bass
import concourse.tile as tile
from concourse import bacc, mybir
from concourse.bass_utils import run_bass_kernel_spmd

F32 = mybir.dt.float32
BF16 = mybir.dt.bfloat16
ALU = mybir.AluOpType
AF = mybir.ActivationFunctionType

B, S, D, H, DK, F = 4, 2048, 256, 8, 32, 1024
Q = S // 2              # queries per core
NST = S // 128          # 16 s-tiles
NQT = Q // 128          # 8 q-tiles
SCALE = float(1.0 / np.sqrt(DK))
RBAR = float(1.0 / S)
LN_EPS = 1e-6

_CACHE = {}
LAST_EXEC_NS = None


def _build():
    nc = bacc.Bacc("TRN2", target_bir_lowering=False, debug=False,
                   num_devices=8)

    # ---- I/O ----
    x_full = nc.dram_tensor("x_full", [S, D], F32, kind="ExternalInput")
    xq_in = nc.dram_tensor("xq", [Q, D], F32, kind="ExternalInput")
    ident_in = nc.dram_tensor("ident", [128, 128], F32, kind="ExternalInput")
    dram_in = {}
    for p in ("g", "l"):
        for t in ("q", "k", "v", "o"):
            dram_in[f"{p}{t}_w"] = nc.dram_tensor(f"{p}{t}_w", [D, D], F32,
                                                  kind="ExternalInput")
            dram_in[f"{p}{t}_b"] = nc.dram_tensor(f"{p}{t}_b", [D], F32,
                                                  kind="ExternalInput")
    for nm, shp, dt in (("ffn_w1", [D, F], F32), ("ffn_b1", [F], F32),
                        ("ffn_w2bf", [F, D], BF16), ("ffn_b2", [D], F32),
                        ("res_w", [D, D], F32), ("res_b", [D], F32),
                        ("ln1_g", [D], F32), ("ln1_b", [D], F32),
                        ("ln2_g", [D], F32), ("ln2_b", [D], F32),
                        ("ln3_g", [D], F32), ("ln3_b", [D], F32),
                        ("bnM", [D], F32), ("bnC", [D], F32)):
        dram_in[nm] = nc.dram_tensor(nm, shp, dt, kind="ExternalInput")
    o_bn = nc.dram_tensor("o_bn", [D, Q], F32, kind="ExternalOutput")
    o_colg = nc.dram_tensor("o_colg", [S], F32, kind="ExternalOutput")
    o_coll = nc.dram_tensor("o_coll", [S], F32, kind="ExternalOutput")

    with tile.TileContext(nc) as tc, ExitStack() as ctx:
        const = ctx.enter_context(tc.tile_pool(name="const", bufs=1))
        acts = ctx.enter_context(tc.tile_pool(name="acts", bufs=1))
        rows = ctx.enter_context(tc.tile_pool(name="rows", bufs=1))
        small = ctx.enter_context(tc.tile_pool(name="small", bufs=2))
        work = ctx.enter_context(tc.tile_pool(name="work", bufs=2))
        epool = ctx.enter_context(tc.tile_pool(name="epool", bufs=2))
        ps = ctx.enter_context(tc.tile_pool(name="ps", bufs=1, space="PSUM"))
        dram = ctx.enter_context(tc.tile_pool(name="dram", bufs=1, space="DRAM"))

        def rows_via_dram(dst_row, src_pn, ntiles):
            """dst_row [1, ntiles*128] <- src_pn [128, ntiles] with s=n*128+p."""
            drs = dram.tile([ntiles * 128], F32, tag="drs", bufs=4)
            nc.gpsimd.dma_start(drs[:].rearrange("(n p) -> p n", p=128), src_pn)
            nc.gpsimd.dma_start(dst_row, drs[:].rearrange("(o s) -> o s", o=1))

        # ---- constants ----
        ident = rows.tile([128, 128], F32, tag="bvb")
        nc.sync.dma_start(ident, ident_in[:, :])
        ones_row = const.tile([1, 128], F32)
        nc.vector.memset(ones_row, 1.0)
        ones_col = const.tile([128, 1], F32)
        nc.vector.memset(ones_col, 1.0)
        ones_row32 = const.tile([33, 128], F32)
        nc.vector.memset(ones_row32[32:33, :], 1.0)

        def load_mat(name, rows_, cols, tag, pool=const, dt=F32):
            t = pool.tile([128, rows_ // 128, cols], dt, tag=tag)
            nc.sync.dma_start(t, dram_in[name].rearrange("(c p) n -> p c n",
                                                         p=128))
            return t

        def load_col(name, n=D, tag=None):
            t = const.tile([128, n // 128], F32, tag=tag or f"c_{name}")
            nc.sync.dma_start(t, dram_in[name].rearrange("(c p) -> p c", p=128))
            return t

        def load_row32(name, n=D, tag=None):
            t = const.tile([33, n], F32, tag=tag or f"r_{name}")
            nc.sync.dma_start(t[32:33, :],
                              dram_in[name].rearrange("(o n) -> o n", o=1))
            return t

        w_sb = {}
        for nm in ("ffn_b1", "ffn_b2", "res_b", "ln1_g", "ln1_b", "ln2_g",
                   "ln2_b", "ln3_g", "ln3_b", "bnM", "bnC"):
            w_sb[nm] = load_col(nm, F if nm == "ffn_b1" else D)
        for nm in ("ln1_g", "ln2_g", "ln3_g"):
            w_sb[nm + "_row"] = load_row32(nm)

        # ---- load x (token-major, staged per tile) + transposes ----
        xq_tok = acts.tile([128, NQT, D], F32, tag="midA")
        nc.sync.dma_start(xq_tok, xq_in.rearrange("(n p) d -> p n d", p=128))

        def transpose_to_feat(tok, ntiles, tag):
            ft = acts.tile([128, 2, ntiles * 128], F32, tag=tag)
            for i in range(ntiles):
                for dch in range(2):
                    pst = ps.tile([128, 128], F32, tag="scps", bufs=2)
                    nc.tensor.transpose(pst, tok[:, i, dch * 128:(dch + 1) * 128],
                                        ident)
                    nc.scalar.activation(ft[:, dch, i * 128:(i + 1) * 128], pst,
                                         AF.Identity)
            return ft

        xT = acts.tile([128, 2, S], F32, tag="bigA")
        xf_stats = small.tile([128, NST, 6], F32, tag="ln_st")
        for i in range(NST):
            stg = work.tile([128, D], F32, tag="xstage")
            nc.sync.dma_start(stg, x_full[i * 128:(i + 1) * 128, :])
            nc.vector.bn_stats(out=xf_stats[:, i, :], in_=stg)
            for dch in range(2):
                pst = ps.tile([128, 128], F32, tag="scps", bufs=2)
                nc.tensor.transpose(pst, stg[:, dch * 128:(dch + 1) * 128], ident)
                nc.scalar.activation(xT[:, dch, i * 128:(i + 1) * 128], pst,
                                     AF.Identity)
        xqT = transpose_to_feat(xq_tok, NQT, "xqT")

        # ---- LN helpers ----
        def rsqrt_dve(v_eps, shape):
            y = small.tile(list(shape), F32, tag="rsq_y")
            a = small.tile(list(shape), F32, tag="rsq_a")
            c = small.tile(list(shape), F32, tag="rsq_c")
            nc.vector.reciprocal(y, v_eps)
            for _ in range(4):
                nc.vector.tensor_tensor(out=a, in0=y, in1=y, op=ALU.mult)
                nc.vector.tensor_tensor(out=a, in0=a, in1=v_eps, op=ALU.mult)
                nc.vector.tensor_scalar(out=c, in0=a, scalar1=-0.5, scalar2=1.5,
                                        op0=ALU.mult, op1=ALU.add)
                nc.vector.tensor_tensor(out=y, in0=y, in1=c, op=ALU.mult)
            return y

        def stats_rows_from_tok(tok, ntiles, pre_stats=None):
            """token-major [128, n, 256] -> mr_rows (row0 rstd, row32 m*rstd)."""
            if pre_stats is None:
                stats = small.tile([128, ntiles, 6], F32, tag="ln_st")
                for i in range(ntiles):
                    nc.vector.bn_stats(out=stats[:, i, :], in_=tok[:, i, :])
            else:
                stats = pre_stats
            mv = small.tile([128, ntiles, 2], F32, tag="ln_mv")
            for i in range(ntiles):
                nc.vector.bn_aggr(out=mv[:, i, :], in_=stats[:, i, :])
            veps = small.tile([128, ntiles], F32, tag="ln_ve")
            nc.vector.tensor_scalar(out=veps, in0=mv[:, :, 1], scalar1=LN_EPS,
                                    scalar2=None, op0=ALU.add)
            rstd = rsqrt_dve(veps, (128, ntiles))
            mrc = small.tile([128, ntiles], F32, tag="ln_mc")
            nc.vector.tensor_tensor(out=mrc, in0=mv[:, :, 0], in1=rstd,
                                    op=ALU.mult)
            mrr = rows.tile([33, ntiles * 128], F32, tag="mr_rows")
            rows_via_dram(mrr[0:1, :], rstd, ntiles)
            rows_via_dram(mrr[32:33, :], mrc, ntiles)
            return mrr

        def stats_rows_from_feat(ft, ntok):
            """feature-major [128, 2, ntok] -> mr_rows [33, ntok]
            (row 0 = rstd, row 32 = m*rstd)."""
            n = ntok // 128
            dsx = dram.tile([ntok], F32, tag="drs", bufs=4)
            dsxx = dram.tile([ntok], F32, tag="drs", bufs=4)
            for j in range(0, ntok, 512):
                pj = ps.tile([1, 512], F32, tag="scps", bufs=2)
                pk = ps.tile([1, 512], F32, tag="pvps", bufs=2)
                for dch in range(2):
                    nc.tensor.matmul(pj, ones_col, ft[:, dch, j:j + 512],
                                     start=(dch == 0), stop=(dch == 1))
                    sq = work.tile([128, 512], F32, tag="ln_sq", bufs=1)
                    nc.vector.tensor_tensor(out=sq, in0=ft[:, dch, j:j + 512],
                                            in1=ft[:, dch, j:j + 512],
                                            op=ALU.mult)
                    nc.tensor.matmul(pk, ones_col, sq,
                                     start=(dch == 0), stop=(dch == 1))
                prow = work.tile([1, 512], F32, tag="prow", bufs=1)
                nc.vector.tensor_copy(prow, pj)
                nc.gpsimd.dma_start(dsx[j:j + 512].rearrange("(o s) -> o s", o=1),
                                    prow)
                prow2 = work.tile([1, 512], F32, tag="prow2", bufs=1)
                nc.vector.tensor_copy(prow2, pk)
                nc.gpsimd.dma_start(dsxx[j:j + 512].rearrange("(o s) -> o s", o=1),
                                    prow2)
            sxr = small.tile([128, n], F32, tag="ln_fst")
            sxxr = small.tile([128, n], F32, tag="ln_fst2")
            nc.gpsimd.dma_start(sxr, dsx[:].rearrange("(n p) -> p n", p=128))
            nc.gpsimd.dma_start(sxxr, dsxx[:].rearrange("(n p) -> p n", p=128))
            m = small.tile([128, n], F32, tag="ln_m2")
            nc.vector.tensor_scalar(out=m, in0=sxr, scalar1=1.0 / D,
                                    scalar2=None, op0=ALU.mult)
            veps = small.tile([128, n], F32, tag="ln_ve")
            msq = small.tile([128, n], F32, tag="ln_msq")
            nc.vector.tensor_tensor(out=msq, in0=m, in1=m, op=ALU.mult)
            nc.vector.scalar_tensor_tensor(out=veps, in0=sxxr,
                                           scalar=1.0 / D, in1=msq,
                                           op0=ALU.mult, op1=ALU.subtract)
            nc.vector.tensor_scalar(out=veps, in0=veps, scalar1=LN_EPS,
                                    scalar2=None, op0=ALU.add)
            rstd = rsqrt_dve(veps, (128, n))
            mrc = small.tile([128, n], F32, tag="ln_mc")
            nc.vector.tensor_tensor(out=mrc, in0=m, in1=rstd, op=ALU.mult)
            mrr = rows.tile([33, ntok], F32, tag="mr_rows")
            rows_via_dram(mrr[0:1, :], rstd, n)
            rows_via_dram(mrr[32:33, :], mrc, n)
            return mrr

        def ln_apply(srcT, ntok, mrr, gkey, tag):
            """out = (x - m) * rstd * g + b, feature-major [128, 2, ntok]."""
            outT = acts.tile([128, 2, ntok], F32, tag=tag)
            g_col = w_sb[gkey + "_g"]
            b_col = w_sb[gkey + "_b"]
            g_row = w_sb[gkey + "_g_row"]
            for dch in range(2):
                for j in range(0, ntok, 512):
                    rb = ps.tile([128, 512], F32, tag="scps", bufs=2)
                    nc.tensor.matmul(rb, ones_row, mrr[0:1, j:j + 512],
                                     start=True, stop=True)
                    gmr = ps.tile([128, 512], F32, tag="pvps", bufs=2)
                    nc.tensor.matmul(gmr,
                                     g_row[32:33, dch * 128:(dch + 1) * 128],
                                     mrr[32:33, j:j + 512],
                                     start=True, stop=True, tile_position=(32, 0))
                    nc.vector.scalar_tensor_tensor(
                        out=outT[:, dch, j:j + 512], in0=srcT[:, dch, j:j + 512],
                        scalar=g_col[:, dch:dch + 1], in1=rb,
                        op0=ALU.mult, op1=ALU.mult)
                    nc.vector.scalar_tensor_tensor(
                        out=outT[:, dch, j:j + 512], in0=outT[:, dch, j:j + 512],
                        scalar=b_col[:, dch:dch + 1], in1=gmr,
                        op0=ALU.add, op1=ALU.subtract)
            return outT

        mr1f = stats_rows_from_tok(None, NST, pre_stats=xf_stats)
        norm1T = ln_apply(xT, S, mr1f, "ln1", "bigB")
        mr1q = stats_rows_from_tok(xq_tok, NQT)
        norm1qT = ln_apply(xqT, Q, mr1q, "ln1", "normqT")

        # ---- projections ----
        def proj_feat(normT, ntok, w_t, b_col, tag, pool=acts):
            out = pool.tile([128, 2, ntok], F32, tag=tag)
            for m in range(2):
                for j in range(0, ntok, 512):
                    psm = ps.tile([128, 512], F32, tag="scps", bufs=2)
                    for c in range(2):
                        nc.tensor.matmul(psm, w_t[:, c, m * 128:(m + 1) * 128],
                                         normT[:, c, j:j + 512],
                                         start=(c == 0), stop=(c == 1))
                    nc.scalar.activation(out[:, m, j:j + 512], psm, AF.Identity,
                                         bias=b_col[:, m:m + 1])
            return out

        def proj_v_aug(normT, w_t, bname):
            """V token-major with ones column: [128, NST, H, DK+2] bf16."""
            vaug = acts.tile([128, NST, H, DK + 2], BF16, tag="tokA")
            nc.vector.memset(vaug[:, :, :, DK:DK + 1], 1.0)
            bvb = rows.tile([128, D], F32, tag="bvb")
            bv_ap = dram_in[bname][:]
            nc.gpsimd.dma_start(
                bvb, bass.AP(tensor=bv_ap.tensor, offset=bv_ap.offset,
                             ap=[[0, 128], [1, D]]))
            for i in range(NST):
                psm = ps.tile([128, D], F32, tag="scps", bufs=2)
                for c in range(2):
                    nc.tensor.matmul(psm, normT[:, c, i * 128:(i + 1) * 128],
                                     w_t[:, c, :], start=(c == 0), stop=(c == 1))
                nc.vector.tensor_tensor(
                    out=vaug[:, i, :, 0:DK],
                    in0=psm.rearrange("p (h k) -> p h k", k=DK),
                    in1=bvb.rearrange("p (h k) -> p h k", k=DK), op=ALU.add)
            return vaug

        # ---- attention ----
        QC = 512

        def mha(normT_full, normT_q, pfx, colsum_dst, res_src, res_dst):
            wq = load_mat(f"{pfx}q_w", D, D, "w_q")
            wk = load_mat(f"{pfx}k_w", D, D, "w_k")
            wv = load_mat(f"{pfx}v_w", D, D, "w_v")
            wo = load_mat(f"{pfx}o_w", D, D, "w_o")
            bq = load_col(f"{pfx}q_b", tag="b_q")
            bk = load_col(f"{pfx}k_b", tag="b_k")
            bo = load_col(f"{pfx}o_b", tag="b_o")
            ktT = proj_feat(normT_full, S, wk, bk, "bigA")
            qtT = proj_feat(normT_q, Q, wq, bq, "bigC")
            vaug = proj_v_aug(normT_full, wv, f"{pfx}v_b")

            colA = small.tile([128, NST], F32, tag="colA")
            colD = small.tile([128, NST], F32, tag="colD")
            otst = acts.tile([128, 2, Q], F32, tag="bigB")

            first = True
            for p in range(H // 2):          # head pairs, band-concurrent
                h0, h1 = 2 * p, 2 * p + 1
                pt = h0 // 4
                b0, b1 = (h0 % 4) * 32, (h1 % 4) * 32
                for qc in range(Q // QC):
                    q0 = qc * QC
                    e_p = epool.tile([128, NST, 2, QC], BF16, tag="e_h")
                    a_p = small.tile([128, NST], F32, tag="a_h")
                    pvps = ps.tile([97, QC], F32, tag="pvps", bufs=2)
                    for st in range(NST):
                        scps = ps.tile([128, 2, QC], F32, tag="scps", bufs=2)
                        nc.tensor.matmul(
                            scps[:, 0, :],
                            ktT[b0:b0 + 32, pt, st * 128:(st + 1) * 128],
                            qtT[b0:b0 + 32, pt, q0:q0 + QC],
                            start=True, stop=True, tile_position=(b0, 0))
                        nc.tensor.matmul(
                            scps[:, 1, :],
                            ktT[b1:b1 + 32, pt, st * 128:(st + 1) * 128],
                            qtT[b1:b1 + 32, pt, q0:q0 + QC],
                            start=True, stop=True, tile_position=(b1, 0))
                        nc.scalar.activation(e_p[:, st, :, :],
                                             scps.rearrange("p h q -> p (h q)"),
                                             AF.Exp, scale=SCALE,
                                             accum_out=a_p[:, st:st + 1])
                        nc.tensor.matmul(pvps[0:DK + 1, :],
                                         vaug[:, st, h0, 0:DK + 1],
                                         e_p[:, st, 0, :],
                                         start=(st == 0), stop=(st == NST - 1))
                        nc.tensor.matmul(pvps[64:64 + DK + 1, :],
                                         vaug[:, st, h1, 0:DK + 1],
                                         e_p[:, st, 1, :],
                                         start=(st == 0), stop=(st == NST - 1))
                    # rowsums -> DRAM -> parallel recip -> DRAM -> broadcasts
                    rsr = work.tile([97, QC], F32, tag="rsr", bufs=1)
                    nc.scalar.activation(rsr[32:33, :], pvps[DK:DK + 1, :],
                                         AF.Identity)
                    nc.scalar.activation(rsr[96:97, :], pvps[64 + DK:64 + DK + 1, :],
                                         AF.Identity)
                    drs_rs = dram.tile([2 * QC], F32, tag="drs", bufs=4)
                    nc.gpsimd.dma_start(
                        drs_rs[0:QC].rearrange("(o s) -> o s", o=1), rsr[32:33, :])
                    nc.gpsimd.dma_start(
                        drs_rs[QC:2 * QC].rearrange("(o s) -> o s", o=1),
                        rsr[96:97, :])
                    rs_par = small.tile([128, 2 * QC // 128], F32, tag="rs_par")
                    nc.gpsimd.dma_start(
                        rs_par, drs_rs[:].rearrange("(n p) -> p n", p=128))
                    r_par = small.tile([128, 2 * QC // 128], F32, tag="r_par")
                    nc.vector.reciprocal(r_par, rs_par)
                    d_par = small.tile([128, 2 * QC // 128], F32, tag="d_par")
                    nc.vector.tensor_scalar(out=d_par, in0=r_par, scalar1=-RBAR,
                                            scalar2=None, op0=ALU.add)
                    drs_r = dram.tile([2 * QC], F32, tag="drs", bufs=4)
                    nc.gpsimd.dma_start(
                        drs_r[:].rearrange("(n p) -> p n", p=128), r_par)
                    drs_d = dram.tile([2 * QC], F32, tag="drs", bufs=4)
                    nc.gpsimd.dma_start(
                        drs_d[:].rearrange("(n p) -> p n", p=128), d_par)
                    rB = work.tile([128, 2, QC], F32, tag="rB", bufs=1)
                    nc.gpsimd.dma_start(
                        rB, bass.AP(tensor=drs_r[:].tensor, offset=drs_r[:].offset,
                                    ap=[[0, 128], [QC, 2], [1, QC]]))
                    dB = work.tile([128, 2, QC], F32, tag="dB", bufs=1)
                    nc.gpsimd.dma_start(
                        dB, bass.AP(tensor=drs_d[:].tensor, offset=drs_d[:].offset,
                                    ap=[[0, 128], [QC, 2], [1, QC]]))
                    # normalized O (via DMA partition-move into otst bands)
                    for hi, (h, bnd) in enumerate(((h0, b0), (h1, b1))):
                        osb = work.tile([DK, QC], F32, tag="osb", bufs=1)
                        nc.scalar.activation(osb, pvps[64 * hi:64 * hi + DK, :],
                                             AF.Identity)
                        otmp = work.tile([DK, QC], F32, tag="otmp", bufs=1)
                        nc.vector.tensor_tensor(out=otmp, in0=osb,
                                                in1=rB[0:DK, hi, :], op=ALU.mult)
                        nc.sync.dma_start(otst[bnd:bnd + 32, pt, q0:q0 + QC],
                                          otmp)
                    # delta colsum over both heads at once
                    scrap = work.tile([128, 2 * QC], BF16, tag="att_scrap",
                                      bufs=1)
                    dcol = small.tile([128, NST], F32, tag="att_dcol")
                    for st in range(NST):
                        nc.vector.scalar_tensor_tensor(
                            out=scrap,
                            in0=e_p[:, st, :, :].rearrange("p h q -> p (h q)"),
                            scalar=1.0,
                            in1=dB.rearrange("p h q -> p (h q)"),
                            op0=ALU.mult, op1=ALU.mult,
                            accum_out=dcol[:, st:st + 1])
                    if first:
                        nc.vector.tensor_copy(colA, a_p)
                        nc.vector.tensor_copy(colD, dcol)
                        first = False
                    else:
                        nc.vector.tensor_tensor(out=colA, in0=colA, in1=a_p,
                                                op=ALU.add)
                        nc.vector.tensor_tensor(out=colD, in0=colD, in1=dcol,
                                                op=ALU.add)

            for m in range(2):
                for j in range(0, Q, 512):
                    psm = ps.tile([128, 512], F32, tag="scps", bufs=2)
                    for c in range(2):
                        nc.tensor.matmul(psm, wo[:, c, m * 128:(m + 1) * 128],
                                         otst[:, c, j:j + 512],
                                         start=(c == 0), stop=(c == 1))
                    nc.vector.scalar_tensor_tensor(
                        out=res_dst[:, m, j:j + 512], in0=psm,
                        scalar=bo[:, m:m + 1], in1=res_src[:, m, j:j + 512],
                        op0=ALU.add, op1=ALU.add)
            colsum = small.tile([128, NST], F32, tag="col_tot")
            nc.vector.tensor_scalar(out=colsum, in0=colA, scalar1=RBAR,
                                    scalar2=None, op0=ALU.mult)
            nc.vector.tensor_tensor(out=colsum, in0=colsum, in1=colD, op=ALU.add)
            nc.sync.dma_start(colsum_dst[:].rearrange("(n p) -> p n", p=128),
                              colsum)

        out1qT = acts.tile([128, 2, Q], F32, tag="midA")
        mha(norm1T, norm1qT, "g", o_colg, xqT, out1qT)

        # ---- AllGather out1 halves ----
        ag_in = dram.tile([D, Q], F32)
        ag_out = dram.tile([2, D, Q], F32)
        nc.sync.dma_start(ag_in.rearrange("(c p) q -> p c q", p=128), out1qT)
        nc.gpsimd.collective_compute(
            "AllGather", ALU.bypass,
            replica_groups=[[0, 1], [2, 3], [4, 5], [6, 7]],
            ins=[ag_in.opt()], outs=[ag_out.opt()])
        out1T = acts.tile([128, 2, S], F32, tag="bigA")
        for half in range(2):
            for dch in range(2):
                nc.sync.dma_start(out1T[:, dch, half * Q:(half + 1) * Q],
                                  ag_out[half, dch * 128:(dch + 1) * 128, :])

        mr2f = stats_rows_from_feat(out1T, S)
        norm2T = ln_apply(out1T, S, mr2f, "ln2", "bigB")
        mr2q = stats_rows_from_feat(out1qT, Q)
        norm2qT = ln_apply(out1qT, Q, mr2q, "ln2", "normqT")

        out2qT = acts.tile([128, 2, Q], F32, tag="bigC2")
        mha(norm2T, norm2qT, "l", o_coll, out1qT, out2qT)

        mr3q = stats_rows_from_feat(out2qT, Q)
        norm3qT = ln_apply(out2qT, Q, mr3q, "ln3", "normqT")

        # ---- FFN ----
        w1 = load_mat("ffn_w1", D, F, "bigA", pool=acts)
        b1 = w_sb["ffn_b1"]
        f1T = acts.tile([128, F // 128, Q], BF16, tag="tokA")
        for m in range(F // 128):
            for j in range(0, Q, 512):
                psm = ps.tile([128, 512], F32, tag="scps", bufs=2)
                for c in range(2):
                    nc.tensor.matmul(psm, w1[:, c, m * 128:(m + 1) * 128],
                                     norm3qT[:, c, j:j + 512],
                                     start=(c == 0), stop=(c == 1))
                nc.scalar.activation(f1T[:, m, j:j + 512], psm, AF.Relu,
                                     bias=b1[:, m:m + 1])
        w2_bf = load_mat("ffn_w2bf", F, D, "w_q", dt=BF16)
        b2 = w_sb["ffn_b2"]
        res_w = load_mat("res_w", D, D, "w_o")
        res_b = w_sb["res_b"]
        for dch in range(2):
            for j in range(0, Q, 512):
                psm = ps.tile([128, 512], F32, tag="scps", bufs=2)
                for c in range(F // 128):
                    nc.tensor.matmul(psm, w2_bf[:, c, dch * 128:(dch + 1) * 128],
                                     f1T[:, c, j:j + 512],
                                     start=(c == 0), stop=(c == F // 128 - 1))
                nc.vector.scalar_tensor_tensor(
                    out=out2qT[:, dch, j:j + 512], in0=psm,
                    scalar=b2[:, dch:dch + 1],
                    in1=out2qT[:, dch, j:j + 512], op0=ALU.add, op1=ALU.add)
                psr = ps.tile([128, 512], F32, tag="pvps", bufs=2)
                for c in range(2):
                    nc.tensor.matmul(psr, res_w[:, c, dch * 128:(dch + 1) * 128],
                                     xqT[:, c, j:j + 512],
                                     start=(c == 0), stop=(c == 1))
                resb = work.tile([128, 512], F32, tag="resb", bufs=1)
                nc.scalar.activation(resb, psr, AF.Identity,
                                     bias=res_b[:, dch:dch + 1])
                nc.vector.tensor_tensor(out=out2qT[:, dch, j:j + 512],
                                        in0=out2qT[:, dch, j:j + 512],
                                        in1=resb, op=ALU.add)
            nc.vector.tensor_scalar(out=out2qT[:, dch, :],
                                    in0=out2qT[:, dch, :],
                                    scalar1=w_sb["bnM"][:, dch:dch + 1],
                                    scalar2=w_sb["bnC"][:, dch:dch + 1],
                                    op0=ALU.mult, op1=ALU.add)
            nc.sync.dma_start(o_bn[dch * 128:(dch + 1) * 128, :],
                              out2qT[:, dch, :])

    nc.compile()
    return nc


def kernel(**inputs):
    global LAST_EXEC_NS
    inp = inputs
    x = np.asarray(inp["x"], np.float32)

    if "nc" not in _CACHE:
        _CACHE["nc"] = _build()
    nc = _CACHE["nc"]

    bnM = (np.asarray(inp["bn_g"], np.float32)
           * (1.0 / np.sqrt(np.asarray(inp["bn_var"], np.float32) + 1e-3))
           ).astype(np.float32)
    bnC = (np.asarray(inp["bn_b"], np.float32)
           - np.asarray(inp["bn_mean"], np.float32) * bnM).astype(np.float32)

    import ml_dtypes
    common = {"ident": np.eye(128, dtype=np.float32), "bnM": bnM, "bnC": bnC}
    for p in ("g", "l"):
        for t in ("q", "k", "v", "o"):
            common[f"{p}{t}_w"] = np.ascontiguousarray(
                np.asarray(inp[f"{p}{t}_w"], np.float32).reshape(D, D))
            common[f"{p}{t}_b"] = np.ascontiguousarray(
                np.asarray(inp[f"{p}{t}_b"], np.float32).reshape(D))
    for nm in ("ffn_w1", "ffn_b1", "ffn_b2", "res_w", "res_b",
               "ln1_g", "ln1_b", "ln2_g", "ln2_b", "ln3_g", "ln3_b"):
        common[nm] = np.ascontiguousarray(np.asarray(inp[nm], np.float32))
    common["ffn_w2bf"] = np.ascontiguousarray(
        np.asarray(inp["ffn_w2"], np.float32).astype(ml_dtypes.bfloat16))

    in_maps = []
    for c in range(8):
        b, half = c // 2, c % 2
        m = dict(common)
        m["x_full"] = np.ascontiguousarray(x[b])
        m["xq"] = np.ascontiguousarray(x[b, half * Q:(half + 1) * Q])
        in_maps.append(m)

    trace = bool(int(os.environ.get("DTB_TRACE", "0")))
    if trace:
        try:
            sys.path.insert(0, "/root/problem/work")
            import ntff_shim
            ntff_shim.install()
        except Exception:
            trace = False
    res = run_bass_kernel_spmd(nc, in_maps, list(range(8)), trace=trace,
                               trace_cores=[0] if trace else None)
    LAST_EXEC_NS = res.exec_time_ns
    _CACHE["last_insts"] = res.instructions_and_trace

    colg = np.zeros((B, S), np.float32)
    coll = np.zeros((B, S), np.float32)
    bn_full = np.zeros((B, S, D), np.float32)
    for c in range(8):
        b, half = c // 2, c % 2
        out = res.results[c]
        colg[b] += out["o_colg"]
        coll[b] += out["o_coll"]
        bn_full[b, half * Q:(half + 1) * Q] = out["o_bn"].T

    imp = (np.float32(0.7 / (H * S)) * colg
           + np.float32(0.3 / (H * S)) * coll).astype(np.float32)
    imp = (imp / (imp.sum(-1, keepdims=True, dtype=np.float32)
                  + np.float32(1e-10))).astype(np.float32)
    si = np.argsort(-imp, axis=-1, kind="stable").astype(np.int32)
    sv = np.take_along_axis(imp, si, axis=-1)
    cum = np.cumsum(sv, axis=-1, dtype=np.float32)
    num = (cum <= np.float32(0.99)).sum(-1)
    min_pts = int(round(S * 0.1))
    num = np.minimum(np.maximum(num, min_pts), S)
    k = int(num.max())
    idx = np.sort(si[:, :k], axis=-1).astype(np.int32)
    down = np.take_along_axis(bn_full, idx[..., None], axis=1)
    return down, idx


# revision 28
# speedup vs baseline: 1.3113x; 1.0047x over previous
"""Trainium2 Bass kernel for nn_DownsampleTransformerBlock.

Self-contained: takes FULL inputs (as from setup_inputs()), shards across 8
NeuronCores (batch x query-half data parallel), runs one SPMD Bass/Tile
kernel, and reassembles the full output (down [B,K,D], idx [B,K]) on host.

Sharding: core c handles batch b=c//2, query half c%2 (1024 queries).
MHA2 needs full-sequence out1 for K/V -> one pairwise AllGather mid-kernel.
Importance colsums: exact-fp32 main term via ACT-accumulate during exp,
plus a small correction term (per-row softmax normalization deviation from
1/S) computed from bf16 attention probs via DVE scalar_tensor_tensor.
"""
import os
import sys
import numpy as np

for _p in ("/opt/trn_rl_repo", "/root/.axon_site"):
    if _p not in sys.path:
        sys.path.insert(0, _p)

from contextlib import ExitStack

import concourse.bass as bass
import concourse.tile as tile
from concourse import bacc, mybir
from concourse.bass_utils import run_bass_kernel_spmd

F32 = mybir.dt.float32
BF16 = mybir.dt.bfloat16
ALU = mybir.AluOpType
AF = mybir.ActivationFunctionType

B, S, D, H, DK, F = 4, 2048, 256, 8, 32, 1024
Q = S // 2              # queries per core
NST = S // 128          # 16 s-tiles
NQT = Q // 128          # 8 q-tiles
SCALE = float(1.0 / np.sqrt(DK))
RBAR = float(1.0 / S)
LN_EPS = 1e-6

_CACHE = {}
LAST_EXEC_NS = None


def _build():
    nc = bacc.Bacc("TRN2", target_bir_lowering=False, debug=False,
                   num_devices=8)

    # ---- I/O ----
    x_full = nc.dram_tensor("x_full", [S, D], F32, kind="ExternalInput")
    xq_in = nc.dram_tensor("xq", [Q, D], F32, kind="ExternalInput")
    ident_in = nc.dram_tensor("ident", [128, 128], F32, kind="ExternalInput")
    dram_in = {}
    for p in ("g", "l"):
        for t in ("q", "k", "v", "o"):
            dram_in[f"{p}{t}_w"] = nc.dram_tensor(f"{p}{t}_w", [D, D], F32,
                                                  kind="ExternalInput")
            dram_in[f"{p}{t}_b"] = nc.dram_tensor(f"{p}{t}_b", [D], F32,
                                                  kind="ExternalInput")
    for nm, shp, dt in (("ffn_w1", [D, F], F32), ("ffn_b1", [F], F32),
                        ("ffn_w2bf", [F, D], BF16), ("ffn_b2", [D], F32),
                        ("res_w", [D, D], F32), ("res_b", [D], F32),
                        ("ln1_g", [D], F32), ("ln1_b", [D], F32),
                        ("ln2_g", [D], F32), ("ln2_b", [D], F32),
                        ("ln3_g", [D], F32), ("ln3_b", [D], F32),
                        ("bnM", [D], F32), ("bnC", [D], F32)):
        dram_in[nm] = nc.dram_tensor(nm, shp, dt, kind="ExternalInput")
    o_bn = nc.dram_tensor("o_bn", [D, Q], F32, kind="ExternalOutput")
    o_colg = nc.dram_tensor("o_colg", [S], F32, kind="ExternalOutput")
    o_coll = nc.dram_tensor("o_coll", [S], F32, kind="ExternalOutput")

    with tile.TileContext(nc) as tc, ExitStack() as ctx:
        const = ctx.enter_context(tc.tile_pool(name="const", bufs=1))
        acts = ctx.enter_context(tc.tile_pool(name="acts", bufs=1))
        rows = ctx.enter_context(tc.tile_pool(name="rows", bufs=1))
        small = ctx.enter_context(tc.tile_pool(name="small", bufs=2))
        work = ctx.enter_context(tc.tile_pool(name="work", bufs=2))
        epool = ctx.enter_context(tc.tile_pool(name="epool", bufs=2))
        ps = ctx.enter_context(tc.tile_pool(name="ps", bufs=1, space="PSUM"))
        dram = ctx.enter_context(tc.tile_pool(name="dram", bufs=1, space="DRAM"))

        def rows_via_dram(dst_row, src_pn, ntiles):
            """dst_row [1, ntiles*128] <- src_pn [128, ntiles] with s=n*128+p."""
            drs = dram.tile([ntiles * 128], F32, tag="drs", bufs=4)
            nc.gpsimd.dma_start(drs[:].rearrange("(n p) -> p n", p=128), src_pn)
            nc.gpsimd.dma_start(dst_row, drs[:].rearrange("(o s) -> o s", o=1))

        # ---- constants ----
        ident = rows.tile([128, 128], F32, tag="bvb")
        nc.sync.dma_start(ident, ident_in[:, :])
        ones_row = const.tile([1, 128], F32)
        nc.vector.memset(ones_row, 1.0)
        ones_col = const.tile([128, 1], F32)
        nc.vector.memset(ones_col, 1.0)
        ones_row32 = const.tile([33, 128], F32)
        nc.vector.memset(ones_row32[32:33, :], 1.0)

        def load_mat(name, rows_, cols, tag, pool=const, dt=F32):
            t = pool.tile([128, rows_ // 128, cols], dt, tag=tag)
            nc.sync.dma_start(t, dram_in[name].rearrange("(c p) n -> p c n",
                                                         p=128))
            return t

        def load_col(name, n=D, tag=None):
            t = const.tile([128, n // 128], F32, tag=tag or f"c_{name}")
            nc.sync.dma_start(t, dram_in[name].rearrange("(c p) -> p c", p=128))
            return t

        def load_row32(name, n=D, tag=None):
            t = const.tile([33, n], F32, tag=tag or f"r_{name}")
            nc.sync.dma_start(t[32:33, :],
                              dram_in[name].rearrange("(o n) -> o n", o=1))
            return t

        w_sb = {}
        for nm in ("ffn_b1", "ffn_b2", "res_b", "ln1_g", "ln1_b", "ln2_g",
                   "ln2_b", "ln3_g", "ln3_b", "bnM", "bnC"):
            w_sb[nm] = load_col(nm, F if nm == "ffn_b1" else D)
        for nm in ("ln1_g", "ln2_g", "ln3_g"):
            w_sb[nm + "_row"] = load_row32(nm)

        # ---- load x (token-major, staged per tile) + transposes ----
        xq_tok = acts.tile([128, NQT, D], F32, tag="midA")
        nc.sync.dma_start(xq_tok, xq_in.rearrange("(n p) d -> p n d", p=128))

        def transpose_to_feat(tok, ntiles, tag):
            ft = acts.tile([128, 2, ntiles * 128], F32, tag=tag)
            for i in range(ntiles):
                for dch in range(2):
                    pst = ps.tile([128, 128], F32, tag="scps", bufs=2)
                    nc.tensor.transpose(pst, tok[:, i, dch * 128:(dch + 1) * 128],
                                        ident)
                    nc.scalar.activation(ft[:, dch, i * 128:(i + 1) * 128], pst,
                                         AF.Identity)
            return ft

        xT = acts.tile([128, 2, S], F32, tag="bigA")
        xf_stats = small.tile([128, NST, 6], F32, tag="ln_st")
        for i in range(NST):
            stg = work.tile([128, D], F32, tag="xstage")
            nc.sync.dma_start(stg, x_full[i * 128:(i + 1) * 128, :])
            nc.vector.bn_stats(out=xf_stats[:, i, :], in_=stg)
            for dch in range(2):
                pst = ps.tile([128, 128], F32, tag="scps", bufs=2)
                nc.tensor.transpose(pst, stg[:, dch * 128:(dch + 1) * 128], ident)
                nc.scalar.activation(xT[:, dch, i * 128:(i + 1) * 128], pst,
                                     AF.Identity)
        xqT = transpose_to_feat(xq_tok, NQT, "xqT")

        # ---- LN helpers ----
        def rsqrt_dve(v_eps, shape):
            y = small.tile(list(shape), F32, tag="rsq_y")
            a = small.tile(list(shape), F32, tag="rsq_a")
            c = small.tile(list(shape), F32, tag="rsq_c")
            nc.vector.reciprocal(y, v_eps)
            for _ in range(4):
                nc.vector.tensor_tensor(out=a, in0=y, in1=y, op=ALU.mult)
                nc.vector.tensor_tensor(out=a, in0=a, in1=v_eps, op=ALU.mult)
                nc.vector.tensor_scalar(out=c, in0=a, scalar1=-0.5, scalar2=1.5,
                                        op0=ALU.mult, op1=ALU.add)
                nc.vector.tensor_tensor(out=y, in0=y, in1=c, op=ALU.mult)
            return y

        def stats_rows_from_tok(tok, ntiles, pre_stats=None):
            """token-major [128, n, 256] -> mr_rows (row0 rstd, row32 m*rstd)."""
            if pre_stats is None:
                stats = small.tile([128, ntiles, 6], F32, tag="ln_st")
                for i in range(ntiles):
                    nc.vector.bn_stats(out=stats[:, i, :], in_=tok[:, i, :])
            else:
                stats = pre_stats
            mv = small.tile([128, ntiles, 2], F32, tag="ln_mv")
            for i in range(ntiles):
                nc.vector.bn_aggr(out=mv[:, i, :], in_=stats[:, i, :])
            veps = small.tile([128, ntiles], F32, tag="ln_ve")
            nc.vector.tensor_scalar(out=veps, in0=mv[:, :, 1], scalar1=LN_EPS,
                                    scalar2=None, op0=ALU.add)
            rstd = rsqrt_dve(veps, (128, ntiles))
            mrc = small.tile([128, ntiles], F32, tag="ln_mc")
            nc.vector.tensor_tensor(out=mrc, in0=mv[:, :, 0], in1=rstd,
                                    op=ALU.mult)
            mrr = rows.tile([33, ntiles * 128], F32, tag="mr_rows")
            rows_via_dram(mrr[0:1, :], rstd, ntiles)
            rows_via_dram(mrr[32:33, :], mrc, ntiles)
            return mrr

        def stats_rows_from_feat(ft, ntok):
            """feature-major [128, 2, ntok] -> mr_rows [33, ntok]
            (row 0 = rstd, row 32 = m*rstd)."""
            n = ntok // 128
            dsx = dram.tile([ntok], F32, tag="drs", bufs=4)
            dsxx = dram.tile([ntok], F32, tag="drs", bufs=4)
            for j in range(0, ntok, 512):
                pj = ps.tile([1, 512], F32, tag="scps", bufs=2)
                pk = ps.tile([1, 512], F32, tag="pvps", bufs=2)
                for dch in range(2):
                    nc.tensor.matmul(pj, ones_col, ft[:, dch, j:j + 512],
                                     start=(dch == 0), stop=(dch == 1))
                    sq = work.tile([128, 512], F32, tag="ln_sq", bufs=1)
                    nc.vector.tensor_tensor(out=sq, in0=ft[:, dch, j:j + 512],
                                            in1=ft[:, dch, j:j + 512],
                                            op=ALU.mult)
                    nc.tensor.matmul(pk, ones_col, sq,
                                     start=(dch == 0), stop=(dch == 1))
                prow = work.tile([1, 512], F32, tag="prow", bufs=1)
                nc.vector.tensor_copy(prow, pj)
                nc.gpsimd.dma_start(dsx[j:j + 512].rearrange("(o s) -> o s", o=1),
                                    prow)
                prow2 = work.tile([1, 512], F32, tag="prow2", bufs=1)
                nc.vector.tensor_copy(prow2, pk)
                nc.gpsimd.dma_start(dsxx[j:j + 512].rearrange("(o s) -> o s", o=1),
                                    prow2)
            sxr = small.tile([128, n], F32, tag="ln_fst")
            sxxr = small.tile([128, n], F32, tag="ln_fst2")
            nc.gpsimd.dma_start(sxr, dsx[:].rearrange("(n p) -> p n", p=128))
            nc.gpsimd.dma_start(sxxr, dsxx[:].rearrange("(n p) -> p n", p=128))
            m = small.tile([128, n], F32, tag="ln_m2")
            nc.vector.tensor_scalar(out=m, in0=sxr, scalar1=1.0 / D,
                                    scalar2=None, op0=ALU.mult)
            veps = small.tile([128, n], F32, tag="ln_ve")
            msq = small.tile([128, n], F32, tag="ln_msq")
            nc.vector.tensor_tensor(out=msq, in0=m, in1=m, op=ALU.mult)
            nc.vector.scalar_tensor_tensor(out=veps, in0=sxxr,
                                           scalar=1.0 / D, in1=msq,
                                           op0=ALU.mult, op1=ALU.subtract)
            nc.vector.tensor_scalar(out=veps, in0=veps, scalar1=LN_EPS,
                                    scalar2=None, op0=ALU.add)
            rstd = rsqrt_dve(veps, (128, n))
            mrc = small.tile([128, n], F32, tag="ln_mc")
            nc.vector.tensor_tensor(out=mrc, in0=m, in1=rstd, op=ALU.mult)
            mrr = rows.tile([33, ntok], F32, tag="mr_rows")
            rows_via_dram(mrr[0:1, :], rstd, n)
            rows_via_dram(mrr[32:33, :], mrc, n)
            return mrr

        def ln_apply(srcT, ntok, mrr, gkey, tag):
            """out = (x - m) * rstd * g + b, feature-major [128, 2, ntok]."""
            outT = acts.tile([128, 2, ntok], F32, tag=tag)
            g_col = w_sb[gkey + "_g"]
            b_col = w_sb[gkey + "_b"]
            g_row = w_sb[gkey + "_g_row"]
            for dch in range(2):
                for j in range(0, ntok, 512):
                    rb = ps.tile([128, 512], F32, tag="scps", bufs=2)
                    nc.tensor.matmul(rb, ones_row, mrr[0:1, j:j + 512],
                                     start=True, stop=True)
                    gmr = ps.tile([128, 512], F32, tag="pvps", bufs=2)
                    nc.tensor.matmul(gmr,
                                     g_row[32:33, dch * 128:(dch + 1) * 128],
                                     mrr[32:33, j:j + 512],
                                     start=True, stop=True, tile_position=(32, 0))
                    nc.vector.scalar_tensor_tensor(
                        out=outT[:, dch, j:j + 512], in0=srcT[:, dch, j:j + 512],
                        scalar=g_col[:, dch:dch + 1], in1=rb,
                        op0=ALU.mult, op1=ALU.mult)
                    nc.vector.scalar_tensor_tensor(
                        out=outT[:, dch, j:j + 512], in0=outT[:, dch, j:j + 512],
                        scalar=b_col[:, dch:dch + 1], in1=gmr,
                        op0=ALU.add, op1=ALU.subtract)
            return outT

        mr1f = stats_rows_from_tok(None, NST, pre_stats=xf_stats)
        norm1T = ln_apply(xT, S, mr1f, "ln1", "bigB")
        mr1q = stats_rows_from_tok(xq_tok, NQT)
        norm1qT = ln_apply(xqT, Q, mr1q, "ln1", "normqT")

        # ---- projections ----
        def proj_feat(normT, ntok, w_t, b_col, tag, pool=acts):
            out = pool.tile([128, 2, ntok], F32, tag=tag)
            for m in range(2):
                for j in range(0, ntok, 512):
                    psm = ps.tile([128, 512], F32, tag="scps", bufs=2)
                    for c in range(2):
                        nc.tensor.matmul(psm, w_t[:, c, m * 128:(m + 1) * 128],
                                         normT[:, c, j:j + 512],
                                         start=(c == 0), stop=(c == 1))
                    nc.scalar.activation(out[:, m, j:j + 512], psm, AF.Identity,
                                         bias=b_col[:, m:m + 1])
            return out

        def proj_v_aug(normT, w_t, bname):
            """V token-major with ones column: [128, NST, H, DK+2] bf16."""
            vaug = acts.tile([128, NST, H, DK + 2], BF16, tag="tokA")
            nc.vector.memset(vaug[:, :, :, DK:DK + 1], 1.0)
            bvb = rows.tile([128, D], F32, tag="bvb")
            bv_ap = dram_in[bname][:]
            nc.gpsimd.dma_start(
                bvb, bass.AP(tensor=bv_ap.tensor, offset=bv_ap.offset,
                             ap=[[0, 128], [1, D]]))
            for i in range(NST):
                psm = ps.tile([128, D], F32, tag="scps", bufs=2)
                for c in range(2):
                    nc.tensor.matmul(psm, normT[:, c, i * 128:(i + 1) * 128],
                                     w_t[:, c, :], start=(c == 0), stop=(c == 1))
                nc.vector.tensor_tensor(
                    out=vaug[:, i, :, 0:DK],
                    in0=psm.rearrange("p (h k) -> p h k", k=DK),
                    in1=bvb.rearrange("p (h k) -> p h k", k=DK), op=ALU.add)
            return vaug

        # ---- attention ----
        QC = 512

        def mha(normT_full, normT_q, pfx, colsum_dst, res_src, res_dst):
            wq = load_mat(f"{pfx}q_w", D, D, "w_q")
            wk = load_mat(f"{pfx}k_w", D, D, "w_k")
            wv = load_mat(f"{pfx}v_w", D, D, "w_v")
            wo = load_mat(f"{pfx}o_w", D, D, "w_o")
            bq = load_col(f"{pfx}q_b", tag="b_q")
            bk = load_col(f"{pfx}k_b", tag="b_k")
            bo = load_col(f"{pfx}o_b", tag="b_o")
            ktT = proj_feat(normT_full, S, wk, bk, "bigA")
            qtT = proj_feat(normT_q, Q, wq, bq, "bigC")
            vaug = proj_v_aug(normT_full, wv, f"{pfx}v_b")

            colA = small.tile([128, NST], F32, tag="colA")
            colD = small.tile([128, NST], F32, tag="colD")
            otst = acts.tile([128, 2, Q], F32, tag="bigB")

            first = True
            for p in range(H // 2):          # head pairs, band-concurrent
                h0, h1 = 2 * p, 2 * p + 1
                pt = h0 // 4
                b0, b1 = (h0 % 4) * 32, (h1 % 4) * 32
                for qc in range(Q // QC):
                    q0 = qc * QC
                    e_p = epool.tile([128, NST, 2, QC], BF16, tag="e_h")
                    a_p = small.tile([128, NST], F32, tag="a_h", bufs=4)
                    pvps = ps.tile([97, QC], F32, tag="pvps", bufs=2)
                    for st in range(NST):
                        scps = ps.tile([128, 2, QC], F32, tag="scps", bufs=2)
                        nc.tensor.matmul(
                            scps[:, 0, :],
                            ktT[b0:b0 + 32, pt, st * 128:(st + 1) * 128],
                            qtT[b0:b0 + 32, pt, q0:q0 + QC],
                            start=True, stop=True, tile_position=(b0, 0))
                        nc.tensor.matmul(
                            scps[:, 1, :],
                            ktT[b1:b1 + 32, pt, st * 128:(st + 1) * 128],
                            qtT[b1:b1 + 32, pt, q0:q0 + QC],
                            start=True, stop=True, tile_position=(b1, 0))
                        nc.scalar.activation(e_p[:, st, :, :],
                                             scps.rearrange("p h q -> p (h q)"),
                                             AF.Exp, scale=SCALE,
                                             accum_out=a_p[:, st:st + 1])
                        nc.tensor.matmul(pvps[0:DK + 1, :],
                                         vaug[:, st, h0, 0:DK + 1],
                                         e_p[:, st, 0, :],
                                         start=(st == 0), stop=(st == NST - 1))
                        nc.tensor.matmul(pvps[64:64 + DK + 1, :],
                                         vaug[:, st, h1, 0:DK + 1],
                                         e_p[:, st, 1, :],
                                         start=(st == 0), stop=(st == NST - 1))
                    # rowsums -> DRAM -> parallel recip -> DRAM -> broadcasts
                    rsr = work.tile([97, QC], F32, tag="rsr", bufs=2)
                    nc.scalar.activation(rsr[32:33, :], pvps[DK:DK + 1, :],
                                         AF.Identity)
                    nc.scalar.activation(rsr[96:97, :], pvps[64 + DK:64 + DK + 1, :],
                                         AF.Identity)
                    drs_rs = dram.tile([2 * QC], F32, tag="drs", bufs=4)
                    nc.gpsimd.dma_start(
                        drs_rs[0:QC].rearrange("(o s) -> o s", o=1), rsr[32:33, :])
                    nc.gpsimd.dma_start(
                        drs_rs[QC:2 * QC].rearrange("(o s) -> o s", o=1),
                        rsr[96:97, :])
                    rs_par = small.tile([128, 2 * QC // 128], F32, tag="rs_par", bufs=4)
                    nc.gpsimd.dma_start(
                        rs_par, drs_rs[:].rearrange("(n p) -> p n", p=128))
                    r_par = small.tile([128, 2 * QC // 128], F32, tag="r_par", bufs=4)
                    nc.vector.reciprocal(r_par, rs_par)
                    d_par = small.tile([128, 2 * QC // 128], F32, tag="d_par", bufs=4)
                    nc.vector.tensor_scalar(out=d_par, in0=r_par, scalar1=-RBAR,
                                            scalar2=None, op0=ALU.add)
                    drs_r = dram.tile([2 * QC], F32, tag="drs", bufs=4)
                    nc.gpsimd.dma_start(
                        drs_r[:].rearrange("(n p) -> p n", p=128), r_par)
                    drs_d = dram.tile([2 * QC], F32, tag="drs", bufs=4)
                    nc.gpsimd.dma_start(
                        drs_d[:].rearrange("(n p) -> p n", p=128), d_par)
                    rB = work.tile([128, 2, QC], F32, tag="rB", bufs=1)
                    nc.gpsimd.dma_start(
                        rB, bass.AP(tensor=drs_r[:].tensor, offset=drs_r[:].offset,
                                    ap=[[0, 128], [QC, 2], [1, QC]]))
                    dB = work.tile([128, 2, QC], F32, tag="dB", bufs=1)
                    nc.gpsimd.dma_start(
                        dB, bass.AP(tensor=drs_d[:].tensor, offset=drs_d[:].offset,
                                    ap=[[0, 128], [QC, 2], [1, QC]]))
                    # normalized O (via DMA partition-move into otst bands)
                    for hi, (h, bnd) in enumerate(((h0, b0), (h1, b1))):
                        osb = work.tile([DK, QC], F32, tag="osb", bufs=1)
                        nc.scalar.activation(osb, pvps[64 * hi:64 * hi + DK, :],
                                             AF.Identity)
                        otmp = work.tile([DK, QC], F32, tag="otmp", bufs=1)
                        nc.vector.tensor_tensor(out=otmp, in0=osb,
                                                in1=rB[0:DK, hi, :], op=ALU.mult)
                        nc.sync.dma_start(otst[bnd:bnd + 32, pt, q0:q0 + QC],
                                          otmp)
                    # delta colsum over both heads at once
                    scrap = work.tile([128, 2 * QC], BF16, tag="att_scrap",
                                      bufs=1)
                    dcol = small.tile([128, NST], F32, tag="att_dcol", bufs=4)
                    for st in range(NST):
                        nc.vector.scalar_tensor_tensor(
                            out=scrap,
                            in0=e_p[:, st, :, :].rearrange("p h q -> p (h q)"),
                            scalar=1.0,
                            in1=dB.rearrange("p h q -> p (h q)"),
                            op0=ALU.mult, op1=ALU.mult,
                            accum_out=dcol[:, st:st + 1])
                    if first:
                        nc.vector.tensor_copy(colA, a_p)
                        nc.vector.tensor_copy(colD, dcol)
                        first = False
                    else:
                        nc.vector.tensor_tensor(out=colA, in0=colA, in1=a_p,
                                                op=ALU.add)
                        nc.vector.tensor_tensor(out=colD, in0=colD, in1=dcol,
                                                op=ALU.add)

            for m in range(2):
                for j in range(0, Q, 512):
                    psm = ps.tile([128, 512], F32, tag="scps", bufs=2)
                    for c in range(2):
                        nc.tensor.matmul(psm, wo[:, c, m * 128:(m + 1) * 128],
                                         otst[:, c, j:j + 512],
                                         start=(c == 0), stop=(c == 1))
                    nc.vector.scalar_tensor_tensor(
                        out=res_dst[:, m, j:j + 512], in0=psm,
                        scalar=bo[:, m:m + 1], in1=res_src[:, m, j:j + 512],
                        op0=ALU.add, op1=ALU.add)
            colsum = small.tile([128, NST], F32, tag="col_tot")
            nc.vector.tensor_scalar(out=colsum, in0=colA, scalar1=RBAR,
                                    scalar2=None, op0=ALU.mult)
            nc.vector.tensor_tensor(out=colsum, in0=colsum, in1=colD, op=ALU.add)
            nc.sync.dma_start(colsum_dst[:].rearrange("(n p) -> p n", p=128),
                              colsum)

        out1qT = acts.tile([128, 2, Q], F32, tag="midA")
        mha(norm1T, norm1qT, "g", o_colg, xqT, out1qT)

        # ---- AllGather out1 halves ----
        ag_in = dram.tile([D, Q], F32)
        ag_out = dram.tile([2, D, Q], F32)
        nc.sync.dma_start(ag_in.rearrange("(c p) q -> p c q", p=128), out1qT)
        nc.gpsimd.collective_compute(
            "AllGather", ALU.bypass,
            replica_groups=[[0, 1], [2, 3], [4, 5], [6, 7]],
            ins=[ag_in.opt()], outs=[ag_out.opt()])
        out1T = acts.tile([128, 2, S], F32, tag="bigA")
        for half in range(2):
            for dch in range(2):
                nc.sync.dma_start(out1T[:, dch, half * Q:(half + 1) * Q],
                                  ag_out[half, dch * 128:(dch + 1) * 128, :])

        mr2f = stats_rows_from_feat(out1T, S)
        norm2T = ln_apply(out1T, S, mr2f, "ln2", "bigB")
        mr2q = stats_rows_from_feat(out1qT, Q)
        norm2qT = ln_apply(out1qT, Q, mr2q, "ln2", "normqT")

        out2qT = acts.tile([128, 2, Q], F32, tag="bigC2")
        mha(norm2T, norm2qT, "l", o_coll, out1qT, out2qT)

        mr3q = stats_rows_from_feat(out2qT, Q)
        norm3qT = ln_apply(out2qT, Q, mr3q, "ln3", "normqT")

        # ---- FFN ----
        w1 = load_mat("ffn_w1", D, F, "bigA", pool=acts)
        b1 = w_sb["ffn_b1"]
        f1T = acts.tile([128, F // 128, Q], BF16, tag="tokA")
        for m in range(F // 128):
            for j in range(0, Q, 512):
                psm = ps.tile([128, 512], F32, tag="scps", bufs=2)
                for c in range(2):
                    nc.tensor.matmul(psm, w1[:, c, m * 128:(m + 1) * 128],
                                     norm3qT[:, c, j:j + 512],
                                     start=(c == 0), stop=(c == 1))
                nc.scalar.activation(f1T[:, m, j:j + 512], psm, AF.Relu,
                                     bias=b1[:, m:m + 1])
        w2_bf = load_mat("ffn_w2bf", F, D, "w_q", dt=BF16)
        b2 = w_sb["ffn_b2"]
        res_w = load_mat("res_w", D, D, "w_o")
        res_b = w_sb["res_b"]
        for dch in range(2):
            for j in range(0, Q, 512):
                psm = ps.tile([128, 512], F32, tag="scps", bufs=2)
                for c in range(F // 128):
                    nc.tensor.matmul(psm, w2_bf[:, c, dch * 128:(dch + 1) * 128],
                                     f1T[:, c, j:j + 512],
                                     start=(c == 0), stop=(c == F // 128 - 1))
                nc.vector.scalar_tensor_tensor(
                    out=out2qT[:, dch, j:j + 512], in0=psm,
                    scalar=b2[:, dch:dch + 1],
                    in1=out2qT[:, dch, j:j + 512], op0=ALU.add, op1=ALU.add)
                psr = ps.tile([128, 512], F32, tag="pvps", bufs=2)
                for c in range(2):
                    nc.tensor.matmul(psr, res_w[:, c, dch * 128:(dch + 1) * 128],
                                     xqT[:, c, j:j + 512],
                                     start=(c == 0), stop=(c == 1))
                resb = work.tile([128, 512], F32, tag="resb", bufs=1)
                nc.scalar.activation(resb, psr, AF.Identity,
                                     bias=res_b[:, dch:dch + 1])
                nc.vector.tensor_tensor(out=out2qT[:, dch, j:j + 512],
                                        in0=out2qT[:, dch, j:j + 512],
                                        in1=resb, op=ALU.add)
            nc.vector.tensor_scalar(out=out2qT[:, dch, :],
                                    in0=out2qT[:, dch, :],
                                    scalar1=w_sb["bnM"][:, dch:dch + 1],
                                    scalar2=w_sb["bnC"][:, dch:dch + 1],
                                    op0=ALU.mult, op1=ALU.add)
            nc.sync.dma_start(o_bn[dch * 128:(dch + 1) * 128, :],
                              out2qT[:, dch, :])

    nc.compile()
    return nc


def kernel(**inputs):
    global LAST_EXEC_NS
    inp = inputs
    x = np.asarray(inp["x"], np.float32)

    if "nc" not in _CACHE:
        _CACHE["nc"] = _build()
    nc = _CACHE["nc"]

    bnM = (np.asarray(inp["bn_g"], np.float32)
           * (1.0 / np.sqrt(np.asarray(inp["bn_var"], np.float32) + 1e-3))
           ).astype(np.float32)
    bnC = (np.asarray(inp["bn_b"], np.float32)
           - np.asarray(inp["bn_mean"], np.float32) * bnM).astype(np.float32)

    import ml_dtypes
    common = {"ident": np.eye(128, dtype=np.float32), "bnM": bnM, "bnC": bnC}
    for p in ("g", "l"):
        for t in ("q", "k", "v", "o"):
            common[f"{p}{t}_w"] = np.ascontiguousarray(
                np.asarray(inp[f"{p}{t}_w"], np.float32).reshape(D, D))
            common[f"{p}{t}_b"] = np.ascontiguousarray(
                np.asarray(inp[f"{p}{t}_b"], np.float32).reshape(D))
    for nm in ("ffn_w1", "ffn_b1", "ffn_b2", "res_w", "res_b",
               "ln1_g", "ln1_b", "ln2_g", "ln2_b", "ln3_g", "ln3_b"):
        common[nm] = np.ascontiguousarray(np.asarray(inp[nm], np.float32))
    common["ffn_w2bf"] = np.ascontiguousarray(
        np.asarray(inp["ffn_w2"], np.float32).astype(ml_dtypes.bfloat16))

    in_maps = []
    for c in range(8):
        b, half = c // 2, c % 2
        m = dict(common)
        m["x_full"] = np.ascontiguousarray(x[b])
        m["xq"] = np.ascontiguousarray(x[b, half * Q:(half + 1) * Q])
        in_maps.append(m)

    trace = bool(int(os.environ.get("DTB_TRACE", "0")))
    if trace:
        try:
            sys.path.insert(0, "/root/problem/work")
            import ntff_shim
            ntff_shim.install()
        except Exception:
            trace = False
    res = run_bass_kernel_spmd(nc, in_maps, list(range(8)), trace=trace,
                               trace_cores=[0] if trace else None)
    LAST_EXEC_NS = res.exec_time_ns
    _CACHE["last_insts"] = res.instructions_and_trace

    colg = np.zeros((B, S), np.float32)
    coll = np.zeros((B, S), np.float32)
    bn_full = np.zeros((B, S, D), np.float32)
    for c in range(8):
        b, half = c // 2, c % 2
        out = res.results[c]
        colg[b] += out["o_colg"]
        coll[b] += out["o_coll"]
        bn_full[b, half * Q:(half + 1) * Q] = out["o_bn"].T

    imp = (np.float32(0.7 / (H * S)) * colg
           + np.float32(0.3 / (H * S)) * coll).astype(np.float32)
    imp = (imp / (imp.sum(-1, keepdims=True, dtype=np.float32)
                  + np.float32(1e-10))).astype(np.float32)
    si = np.argsort(-imp, axis=-1, kind="stable").astype(np.int32)
    sv = np.take_along_axis(imp, si, axis=-1)
    cum = np.cumsum(sv, axis=-1, dtype=np.float32)
    num = (cum <= np.float32(0.99)).sum(-1)
    min_pts = int(round(S * 0.1))
    num = np.minimum(np.maximum(num, min_pts), S)
    k = int(num.max())
    idx = np.sort(si[:, :k], axis=-1).astype(np.int32)
    down = np.take_along_axis(bn_full, idx[..., None], axis=1)
    return down, idx
